# revision 43
# baseline (speedup 1.0000x reference)
"""Trainium2 Bass SPMD kernel: 16-head MHA (B=4, S=2048, D=1024), fp32.

Sharding: 8 cores = 4 batches x 2 head-groups (8 heads each). Host pre-
transposes activations/weights into DMA-friendly pre-tiled layouts
([partition][chunk][k-sub][cols], 4-8KB contiguous per partition per
descriptor), so the device never transposes anything:

  - Q/K projections produce QT/KT in [d_local, S] layout (head dim on
    partitions) which directly feeds the scores matmul.
  - Scores are computed transposed ([t, s] in PSUM), exp'd on ScalarE
    (scale=1/8 folded in, no max-subtraction: scores*0.125 max ~10, exp
    ~3e4, fine in fp32), written to SBUF as bf16.
  - Each score group uses separate a/b PSUM+SBUF tiles so every
    scores->exp->PV dependency is tile-atomic (shared-tile subtile deps
    coalesced into spurious cross-group waits that serialized slots).
  - V is produced in natural [t, d] layout with an appended ones column, so
    the PV matmul yields both the unnormalized output (rows 0..63) and the
    softmax denominator (row 64) in one pass. PV psums alternate banks
    e,o,e,o so accumulate RMW turnarounds mostly hide.
  - Normalization: both heads' denominators share partition 64 of one
    [P,2,SC] psum, so the LN / EXP(-x) / raw-copy each run as single ops;
    1/denom broadcast via K=1 ones-matmul + one DVE multiply per head.
  - O-projection contracts attn^T [d_local, s] tiles against w_o columns;
    per-core bf16 partial outputs are summed (+b_o) in fp32 on the host.

Schedule: flat software pipeline over (chunk, head-pair, t-group) slots.
PVs of two groups flush together on alternating slots and background
projection work (next-chunk Q, prev-chunk O, V) bursts on the others --
matmul type switches cost the PE ~120ns of pipeline turnaround, so
same-type runs are batched. PSUM budget (8 banks) is exactly: scores a/b
2+2, PV accumulators 2, proj/bcast scratch 2.
"""
import collections

import ml_dtypes
import numpy as np

import concourse.bass as bass
import concourse.mybir as mybir
from concourse.tile import TileContext
from concourse.bass_utils import run_bass_kernel_spmd

F32 = mybir.dt.float32
F32R = mybir.dt.float32r
BF16 = mybir.dt.bfloat16
AF = mybir.ActivationFunctionType

B, S, D = 4, 2048, 1024
H, DH = 16, 64
HL = 8        # heads per core
DL = HL * DH  # 512 local model dims
P = 128
SC = 512      # s-chunk width
NCH = S // SC  # 4 s-chunks
ND = D // P    # 8 contraction subtiles for D
NM = DL // P   # 4 m-tiles of local outputs
NT = S // P    # 16 t-tiles

_MAX_WAITS = 1
INTERLEAVE = True


def _split_excess_waits(nc, max_waits=_MAX_WAITS):
    """walrus here rejects >1 sync-wait per instruction; spill extras onto
    same-engine NoOps inserted before the instruction."""
    f = nc.m.functions[0]
    n = 0
    for bb in f.blocks:
        changed = False
        out = []
        for inst in bb.instructions:
            si = inst.sync_info
            if si is not None and len(si.on_wait) > max_waits:
                waits = list(si.on_wait)
                keep = waits[-max_waits:]
                spill = waits[:-max_waits]
                for i in range(0, len(spill), max_waits):
                    nop = mybir.InstNoOp(name=f"WSPILL-{n}", ins=[], outs=[])
                    n += 1
                    nop.engine = inst.engine
                    nop.sync_info = mybir.SyncInfo(
                        on_wait=spill[i : i + max_waits], on_update=[]
                    )
                    nc.register_instruction(nop, overwrite=True)
                    out.append(nop)
                inst.sync_info = mybir.SyncInfo(
                    on_wait=keep, on_update=list(si.on_update)
                )
                changed = True
            out.append(inst)
        if changed:
            bb.instructions = out
    return n


def build():
    nc = bass.Bass()
    # x/w arrive pre-tiled from the host ([partition][chunk][k-sub][cols])
    # so every DMA descriptor is 4-8KB contiguous per partition instead of
    # the 1KB slivers a strided rearrange would produce (~3x DMA speedup
    # on the 16MB input stream; the startup matmul gate is DMA-bound).
    xq = nc.dram_tensor("xq", [P, NCH, ND, SC], BF16, kind="ExternalInput")
    xk = nc.dram_tensor("xk", [P, NCH, ND, SC], BF16, kind="ExternalInput")
    xv = nc.dram_tensor("xv", [P, NCH, ND, SC], BF16, kind="ExternalInput")
    wq = nc.dram_tensor("wq", [P, ND, DL], BF16, kind="ExternalInput")
    wk = nc.dram_tensor("wk", [P, ND, DL], BF16, kind="ExternalInput")
    wv = nc.dram_tensor("wv", [P, ND, DL], BF16, kind="ExternalInput")
    wo = nc.dram_tensor("wo", [P, NM, D], BF16, kind="ExternalInput")
    bq = nc.dram_tensor("bq", [DL], F32, kind="ExternalInput")
    bk = nc.dram_tensor("bk", [DL], F32, kind="ExternalInput")
    bv = nc.dram_tensor("bv", [DL], F32R, kind="ExternalInput")
    out = nc.dram_tensor("out", [S, D], BF16, kind="ExternalOutput")

    xq_r, xk_r, xv_r = xq, xk, xv
    wq_r, wk_r, wv_r, wo_r = wq, wk, wv, wo
    out_r = out.rearrange("(so p) n -> p so n", p=P)

    with TileContext(nc) as tc:
        with (
            tc.tile_pool(name="persist", bufs=1) as persist,
            tc.tile_pool(name="wpool", bufs=3) as wpool,
            tc.tile_pool(name="xpool", bufs=10) as xpool,
            tc.tile_pool(name="expp", bufs=4) as expp,
            tc.tile_pool(name="attnp", bufs=2) as attnp,
            tc.tile_pool(name="osb", bufs=2) as osbp,
            tc.tile_pool(name="nrm", bufs=2) as nrm,
            tc.tile_pool(name="ps_big", bufs=1, space="PSUM") as ps_big,
            tc.tile_pool(name="ps_pv", bufs=1, space="PSUM") as ps_pv,
            tc.tile_pool(name="ps_sm", bufs=2, space="PSUM") as ps_sm,
        ):
            qt = persist.tile([P, NM, S], BF16, tag="qt")
            kt = persist.tile([P, NM, S], BF16, tag="kt")
            vaug = persist.tile([P, NT, HL, 66], BF16, tag="vaug")
            wq_t = persist.tile([P, ND, DL], BF16, tag="wq")
            ones_f = persist.tile([P, P], F32, tag="ones_f")
            ones_r = persist.tile([P, P], F32R, tag="ones_r")
            bq_sb = persist.tile([P, NM], F32, tag="bq")
            bk_sb = persist.tile([P, NM], F32, tag="bk")
            bv_t = persist.tile([P, DL], F32R, tag="bv_t")
            bv_bc = persist.tile([P, DL], F32, tag="bv_bc")

            # ---- constants / biases ----
            nc.vector.memset(ones_f[:], 1.0)
            nc.vector.tensor_copy(ones_r[:], ones_f[:])
            nc.sync.dma_start(bq_sb[:], bq.rearrange("(o p) -> p o", p=P))
            nc.sync.dma_start(bk_sb[:], bk.rearrange("(o p) -> p o", p=P))
            nc.sync.dma_start(bv_t[0:1, :], bv[None, :])
            ps = ps_sm.tile([P, SC], F32, tag="sm")
            nc.tensor.matmul(ps[:], ones_r[0:1, 0:P], bv_t[0:1, :], start=True,
                             stop=True)
            nc.vector.tensor_copy(bv_bc[:], ps[:])
            # ones columns of V_aug
            of = ones_f[:, 0:NT * HL].rearrange("p (a b) -> p a b", a=NT)
            nc.vector.tensor_copy(vaug[:, :, :, 0:1], of[:, :, :, None])
            nc.vector.tensor_copy(vaug[:, :, :, 65:66], of[:, :, :, None])

            def load_x_chunk(x_r, c):
                xa = xpool.tile([P, ND // 2, SC], BF16, tag="x")
                xb = xpool.tile([P, ND // 2, SC], BF16, tag="x")
                nc.sync.dma_start(xa[:], x_r[:, c, 0 : ND // 2, :])
                nc.sync.dma_start(xb[:], x_r[:, c, ND // 2 : ND, :])
                return xa, xb

            def proj_chunk_steps(dst, w_tile, x_r, c, bias_sb, xs=None):
                """Generator: each step emits ~2 matmuls (or 2 bias adds) so
                the attention loop can trickle projection work into its
                per-group PE slack instead of a blocking burst."""
                halves = xs if xs is not None else load_x_chunk(x_r, c)
                # pairs of m-tiles with interleaved k-chains: consecutive
                # matmuls accumulate into different psum banks (no RMW stall)
                for i in range(NM // 2):
                    psms = (ps_sm.tile([P, SC], F32, tag="sm", name="psm0"),
                            ps_sm.tile([P, SC], F32, tag="sm", name="psm1"))
                    for k in range(ND):
                        for h, psm in enumerate(psms):
                            m = 2 * i + h
                            nc.tensor.matmul(
                                psm[:],
                                w_tile[:, k, m * P : (m + 1) * P],
                                halves[k // 4][:, k % 4, :],
                                start=(k == 0),
                                stop=(k == ND - 1),
                            )
                        yield
                    for h, psm in enumerate(psms):
                        m = 2 * i + h
                        nc.vector.tensor_add(
                            dst[:, m, c * SC : (c + 1) * SC],
                            psm[:],
                            bias_sb[:, m : m + 1].to_broadcast((P, SC)),
                        )
                    yield

            def proj_chunk(dst, w_tile, x_r, c, bias_sb, xs=None):
                for _ in proj_chunk_steps(dst, w_tile, x_r, c, bias_sb, xs):
                    pass

            def vproj_chunk_steps(wv_t, c, xa, xb):
                halves = (xa, xb)
                for i2 in range(2):
                    psms = (ps_sm.tile([P, SC], F32, tag="sm", name="psm0"),
                            ps_sm.tile([P, SC], F32, tag="sm", name="psm1"))
                    for k in range(ND):
                        for h2, psm in enumerate(psms):
                            i = 2 * i2 + h2
                            nc.tensor.matmul(
                                psm[:],
                                halves[k // 4][:, k % 4, i * P : (i + 1) * P],
                                wv_t[:, k, :],
                                start=(k == 0),
                                stop=(k == ND - 1),
                            )
                        yield
                    for h2, psm in enumerate(psms):
                        t_o = c * 4 + 2 * i2 + h2
                        for h in range(HL):
                            nc.vector.tensor_add(
                                vaug[:, t_o, h, 1:65],
                                psm[:, h * DH : (h + 1) * DH],
                                bv_bc[:, h * DH : (h + 1) * DH],
                            )
                    yield

            def normalize_pre(pv):
                """Consume the PV psum right away (frees the psum slot): copy
                unnormalized rows on DVE; 1/denom = exp(-ln(denom)) on ACT
                (DVE reciprocal is an 8-pass iterative divide; ln and exp
                share one act table set so these cost ~1.1us each). pv is
                [P, 2, SC] (banks e/o); both denoms sit on partition 64 so
                the copy/ln/exp each run as ONE op over both banks."""
                raw = nrm.tile([P, 2, SC], F32, tag="raw")
                rec = nrm.tile([P, 2, SC], F32R, tag="rec")
                nc.vector.tensor_copy(raw[0:64, :, :], pv[0:64, :, :])
                nc.scalar.activation(raw[64:65, :, :], pv[64:65, :, :], AF.Ln)
                nc.scalar.activation(rec[64:65, :, :], raw[64:65, :, :],
                                     AF.Exp, scale=-1.0)
                return raw, rec

            def normalize_post(raw, rec, p, att):
                """Broadcast 1/denom across partitions (K=1 matmul) and apply."""
                bc_e = ps_sm.tile([P, SC], F32, tag="sm")
                nc.tensor.matmul(bc_e[0:64, :], ones_r[64:65, 0:64],
                                 rec[64:65, 0, :], start=True, stop=True)
                nc.vector.tensor_mul(att[0:64, p, :], bc_e[0:64, :],
                                     raw[0:64, 0, :])
                bc_o = ps_sm.tile([P, SC], F32, tag="sm")
                nc.tensor.matmul(bc_o[0:64, :], ones_r[64:65, 0:64],
                                 rec[64:65, 1, :], start=True, stop=True)
                tmp = nrm.tile([P, SC], BF16, tag="tmp")
                nc.vector.tensor_mul(tmp[0:64, :], bc_o[0:64, :],
                                     raw[0:64, 1, :])
                # scalar-DGE queue: keeps this small partition-shift transfer
                # from queueing behind megabyte output DMAs on the SP queue
                # (it gates the next chunk's o-proj start)
                nc.scalar.dma_start(att[64:128, p, :], tmp[0:64, :])

            # ---- projections (prefix) ----
            # q-chunk-0 x DMAs lead the queue (they gate the very first
            # matmul); weight DMAs follow (wpool bufs=3 keeps k/v/o
            # resident together; bf16 makes that cheap).
            xq0a = xpool.tile([P, ND // 2, SC], BF16, tag="x", name="xq0a")
            xq0b = xpool.tile([P, ND // 2, SC], BF16, tag="x", name="xq0b")
            nc.sync.dma_start(xq0a[:], xq_r[:, 0, 0 : ND // 2, :])
            nc.sync.dma_start(wq_t[:, 0 : ND // 2], wq_r[:, 0 : ND // 2])
            nc.sync.dma_start(xq0b[:], xq_r[:, 0, ND // 2 : ND, :])
            nc.sync.dma_start(wq_t[:, ND // 2 :], wq_r[:, ND // 2 :])
            xq0 = (xq0a, xq0b)
            wk_t = wpool.tile([P, ND, DL], BF16, tag="w")
            nc.sync.dma_start(wk_t[:, 0 : ND // 2], wk_r[:, 0 : ND // 2])
            nc.sync.dma_start(wk_t[:, ND // 2 :], wk_r[:, ND // 2 :])
            # all xk chunks queue before wv/wo (k-proj consumes them long
            # before V/O weights are touched)
            kx = [load_x_chunk(xk_r, c) for c in range(NCH)]
            proj_chunk(qt, wq_t, xq_r, 0, bq_sb, xs=xq0)
            wv_t = wpool.tile([P, ND, DL], BF16, tag="w")
            nc.sync.dma_start(wv_t[:], wv_r[:])
            wo_t = wpool.tile([P, NM, D], BF16, tag="w")
            nc.sync.dma_start(wo_t[:], wo_r[:])
            for c in range(NCH):
                proj_chunk(kt, wk_t, xk_r, c, bk_sb, xs=kx[c])
            # issue all xv DMAs now (xpool bufs=10 holds them) so no vproj
            # step ever head-of-line blocks the PE on a transfer
            vx = [load_x_chunk(xv_r, c) for c in range(NCH)]
            vproj_gens = [
                vproj_chunk_steps(wv_t, c, *vx[c]) for c in range(NCH)
            ]

            # ---- attention + o-proj, per s-chunk ----
            def oproj_chunk_steps(c, attn_t):
                for st in range(4):
                    psms = (ps_sm.tile([P, SC], F32, tag="sm", name="psm0"),
                            ps_sm.tile([P, SC], F32, tag="sm", name="psm1"))
                    for do in range(NM):
                        for n, psm in enumerate(psms):
                            nc.tensor.matmul(
                                psm[:],
                                attn_t[:, do, st * P : (st + 1) * P],
                                wo_t[:, do, n * SC : (n + 1) * SC],
                                start=(do == 0),
                                stop=(do == NM - 1),
                            )
                        yield
                    for n, psm in enumerate(psms):
                        ob = osbp.tile([P, SC], BF16, tag="ob")
                        nc.vector.tensor_copy(ob[:], psm[:])
                        nc.sync.dma_start(
                            out_r[:, c * 4 + st, n * SC : (n + 1) * SC], ob[:]
                        )
                    yield

            def oproj_chunk(c, attn_t):
                for _ in oproj_chunk_steps(c, attn_t):
                    pass

            def emit_pv_half(pend, half):
                # consecutive PVs alternate psum banks (e,o,e,o) so the
                # accumulate RMW turnaround never hits a back-to-back bank
                ex_a, ex_b, ta, tb, p, pv, att = pend
                ex, t = (ex_a, ta) if half == 0 else (ex_b, tb)
                for hh, bank in ((2 * p, 0), (2 * p + 1, 1)):
                    nc.tensor.matmul(
                        pv[0:65, bank, :],
                        vaug[:, t, hh, 1:66],
                        ex[:, hh % 2, :],
                        start=(t == 0),
                        stop=(t == NT - 1),
                    )

            def finish_pend(pend):
                """After both PV halves are out: when that group was its p's
                last, normalize and return the deferred normalize_post
                closure for that p."""
                ex_a, ex_b, ta, tb, p, pv, att = pend
                if tb != NT - 1:
                    return None
                raw, rec = normalize_pre(pv)

                def run(raw=raw, rec=rec, p=p, att=att):
                    normalize_post(raw, rec, p, att)

                return run

            def flush_pend(pend):
                emit_pv_half(pend, 0)
                emit_pv_half(pend, 1)
                return finish_pend(pend)

            # flat software pipeline across (c, p, g): `pend` (the PV work
            # for the previous score group) carries across p- and c-
            # boundaries, so the PE never drains while a p finishes up
            # (PV flush + normalize happen AFTER the next p's first scores).
            # Projection work (next-chunk qproj, prev-chunk oproj) trickles
            # in as ~2-matmul background steps per group slot instead of
            # multi-us bursts that would stall the scores->exp chain.
            bg = collections.deque()

            def bg_pull(steps=None):
                if steps is None:
                    steps = 2 if len(bg) > 1 else 1
                while bg and steps > 0:
                    try:
                        next(bg[0])
                        steps -= 1
                    except StopIteration:
                        bg.popleft()

            for gen in vproj_gens:
                bg.append(gen)
            pends = collections.deque()
            deferred = None
            prev_attn = None
            for c in range(NCH):
                attn_t = attnp.tile([P, NM, SC], BF16, tag="attn")
                cs = slice(c * SC, (c + 1) * SC)
                for p in range(NM):
                    pv = ps_pv.tile([P, 2, SC], F32, tag="pv")
                    # during (c0, p0) V-proj streams in via bg; delay PV
                    # flushing by one extra group so each PV's vaug tiles
                    # are already emitted ahead of it in PE program order.
                    warmup = c == 0 and p == 0
                    depth = 2 if warmup else 1
                    for g in range(NT // 2):
                        ta, tb = 2 * g, 2 * g + 1
                        # separate a/b tiles so every scores->exp->PV dep is
                        # tile-atomic: shared-tile subtile deps coalesced into
                        # cross-group semaphore waits (PV(g) stalling on
                        # exp#1(g+1)) that serialized the whole slot.
                        big_a = ps_big.tile([P, 2, SC], F32, tag="biga")
                        big_b = ps_big.tile([P, 2, SC], F32, tag="bigb")
                        ex_a = expp.tile([P, 2, SC], BF16, tag="exa")
                        ex_b = expp.tile([P, 2, SC], BF16, tag="exb")
                        flush = None
                        if INTERLEAVE:
                            # slot order [s0 s1][PVa][s2 s3][PVb]: the prior
                            # group's PV halves sit between this group's
                            # score pairs so same-bank PV accumulates land
                            # 3+ instructions apart (RMW turnaround hides)
                            while len(pends) > depth:
                                prev = pends.popleft()
                                if flush is not None:
                                    d = flush_pend(flush)
                                    if d is not None:
                                        deferred = d
                                flush = prev
                        for h, lo in enumerate((0, 64)):
                            nc.tensor.matmul(
                                big_a[:, h, :],
                                kt[lo : lo + 64, p, ta * P : (ta + 1) * P],
                                qt[lo : lo + 64, p, cs],
                                start=True, stop=True,
                            )
                        nc.scalar.activation(ex_a[:], big_a[:], AF.Exp,
                                             scale=0.125)
                        if flush is not None:
                            emit_pv_half(flush, 0)
                        for h, lo in enumerate((0, 64)):
                            nc.tensor.matmul(
                                big_b[:, h, :],
                                kt[lo : lo + 64, p, tb * P : (tb + 1) * P],
                                qt[lo : lo + 64, p, cs],
                                start=True, stop=True,
                            )
                        nc.scalar.activation(ex_b[:], big_b[:], AF.Exp,
                                             scale=0.125)
                        if flush is not None:
                            emit_pv_half(flush, 1)
                            d = finish_pend(flush)
                            if d is not None:
                                deferred = d
                        if not INTERLEAVE and len(pends) >= depth + 1:
                            # flush two groups' PVs together on even slots,
                            # bg-proj bursts on odd slots: every matmul-type
                            # switch (scores<->PV<->proj weight geometry)
                            # costs the PE ~120ns of pipeline turnaround, so
                            # batch same-type runs
                            while pends:
                                d = flush_pend(pends.popleft())
                                if d is not None:
                                    deferred = d
                        if warmup:
                            # one V-proj pair (2 t-tiles) per slot = 9 steps,
                            # emitted before the PV that will consume it
                            for _ in range(9):
                                bg_pull()
                        if g == 3 and deferred is not None:
                            deferred()
                            deferred = None
                        if g == 5 and p == 0 and prev_attn is not None:
                            bg.append(oproj_chunk_steps(c - 1, prev_attn))
                            prev_attn = None
                        if g == 0 and p == 1 and c < NCH - 1:
                            bg.append(proj_chunk_steps(qt, wq_t, xq_r, c + 1,
                                                       bq_sb))
                        pends.append((ex_a, ex_b, ta, tb, p, pv, attn_t))
                        if not warmup:
                            if INTERLEAVE:
                                bg_pull(2 if len(bg) > 1 else 1)
                            elif g % 2 == 1:
                                # deeper bursts while the queue is long:
                                # ACT-bound slots have PE slack to absorb
                                # extra proj work
                                bg_pull(3 if len(bg) > 1 else 2)
                prev_attn = attn_t
            while bg:
                bg_pull()
            while pends:
                d = flush_pend(pends.popleft())
                if d is not None:
                    if deferred is not None:
                        deferred()
                    deferred = d
            deferred()
            oproj_chunk(NCH - 1, prev_attn)

    _split_excess_waits(nc)
    return nc


_CACHE = {}


def _get_nc():
    if "nc" not in _CACHE:
        _CACHE["nc"] = build()
    return _CACHE["nc"]


def _f32(x):
    return np.asarray(x).astype(np.float32, copy=False)


def _tile_x(xt):
    # [D, S] -> [P, NCH, ND, SC]: per-(partition, chunk) contiguous 4KB runs
    return np.ascontiguousarray(
        xt.reshape(ND, P, NCH, SC).transpose(1, 2, 0, 3)
    )


def _tile_w(wt):
    # [D, M] -> [P, ND, M]
    return np.ascontiguousarray(
        wt.reshape(wt.shape[0] // P, P, wt.shape[1]).transpose(1, 0, 2)
    )


def _prep_core_inputs(c, q, k, v, w_q, b_q, w_k, b_k, w_v, b_v, w_o, b_o):
    b, hg = c // 2, c % 2
    hs = slice(hg * DL, hg * DL + DL)
    bf = ml_dtypes.bfloat16
    return {
        "xq": _tile_x(q[b].T.astype(bf)),
        "xk": _tile_x(k[b].T.astype(bf)),
        "xv": _tile_x(v[b].T.astype(bf)),
        "wq": _tile_w(w_q[hs, :].T.astype(bf)),
        "wk": _tile_w(w_k[hs, :].T.astype(bf)),
        "wv": _tile_w(w_v[hs, :].T.astype(bf)),
        "wo": _tile_w(w_o[:, hs].T.astype(bf)),
        "bq": np.ascontiguousarray(b_q[hs]),
        "bk": np.ascontiguousarray(b_k[hs]),
        "bv": np.ascontiguousarray(b_v[hs]),
    }


def kernel(q, k, v, w_q, b_q, w_k, b_k, w_v, b_v, w_o, b_o):
    q, k, v = _f32(q), _f32(k), _f32(v)
    w_q, b_q = _f32(w_q), _f32(b_q)
    w_k, b_k = _f32(w_k), _f32(b_k)
    w_v, b_v = _f32(w_v), _f32(b_v)
    w_o, b_o = _f32(w_o), _f32(b_o)

    nc = _get_nc()
    in_maps = [
        _prep_core_inputs(c, q, k, v, w_q, b_q, w_k, b_k, w_v, b_v, w_o, b_o)
        for c in range(8)
    ]
    res = run_bass_kernel_spmd(nc, in_maps, core_ids=list(range(8)))
    out = np.empty((B, S, D), np.float32)
    for b in range(B):
        out[b] = (
            res.results[2 * b]["out"].astype(np.float32)
            + res.results[2 * b + 1]["out"].astype(np.float32)
            + b_o
        )
    return out



# revision 44
# speedup vs baseline: 1.0226x; 1.0226x over previous
"""Trainium2 Bass SPMD kernel: 16-head MHA (B=4, S=2048, D=1024), fp32.

Sharding: 8 cores = 4 batches x 2 head-groups (8 heads each). Host pre-
transposes activations/weights into DMA-friendly pre-tiled layouts
([partition][chunk][k-sub][cols], 4-8KB contiguous per partition per
descriptor), so the device never transposes anything:

  - Q/K projections produce QT/KT in [d_local, S] layout (head dim on
    partitions) which directly feeds the scores matmul.
  - Scores are computed transposed ([t, s] in PSUM), exp'd on ScalarE
    (scale=1/8 folded in, no max-subtraction: scores*0.125 max ~10, exp
    ~3e4, fine in fp32), written to SBUF as bf16.
  - Each score group uses separate a/b PSUM+SBUF tiles so every
    scores->exp->PV dependency is tile-atomic (shared-tile subtile deps
    coalesced into spurious cross-group waits that serialized slots).
  - V is produced in natural [t, d] layout with an appended ones column, so
    the PV matmul yields both the unnormalized output (rows 0..63) and the
    softmax denominator (row 64) in one pass. PV psums alternate banks
    e,o,e,o so accumulate RMW turnarounds mostly hide.
  - Normalization: both heads' denominators share partition 64 of one
    [P,2,SC] psum, so the LN / EXP(-x) / raw-copy each run as single ops;
    1/denom broadcast via K=1 ones-matmul + one DVE multiply per head.
  - O-projection contracts attn^T [d_local, s] tiles against w_o columns;
    per-core bf16 partial outputs are summed (+b_o) in fp32 on the host.

Schedule: flat software pipeline over (chunk, head-pair, t-group) slots.
PVs of two groups flush together on alternating slots and background
projection work (next-chunk Q, prev-chunk O, V) bursts on the others --
matmul type switches cost the PE ~120ns of pipeline turnaround, so
same-type runs are batched. PSUM budget (8 banks) is exactly: scores a/b
2+2, PV accumulators 2, proj/bcast scratch 2.
"""
import collections

import ml_dtypes
import numpy as np

import concourse.bass as bass
import concourse.mybir as mybir
from concourse.tile import TileContext
from concourse.bass_utils import run_bass_kernel_spmd

F32 = mybir.dt.float32
F32R = mybir.dt.float32r
BF16 = mybir.dt.bfloat16
AF = mybir.ActivationFunctionType

B, S, D = 4, 2048, 1024
H, DH = 16, 64
HL = 8        # heads per core
DL = HL * DH  # 512 local model dims
P = 128
SC = 512      # s-chunk width
NCH = S // SC  # 4 s-chunks
ND = D // P    # 8 contraction subtiles for D
NM = DL // P   # 4 m-tiles of local outputs
NT = S // P    # 16 t-tiles

_MAX_WAITS = 1
INTERLEAVE = False  # measured: interleaving PV halves between score pairs
# loses ~8us to extra matmul-type switches vs. batched pair-flush


def _split_excess_waits(nc, max_waits=_MAX_WAITS):
    """walrus here rejects >1 sync-wait per instruction; spill extras onto
    same-engine NoOps inserted before the instruction."""
    f = nc.m.functions[0]
    n = 0
    for bb in f.blocks:
        changed = False
        out = []
        for inst in bb.instructions:
            si = inst.sync_info
            if si is not None and len(si.on_wait) > max_waits:
                waits = list(si.on_wait)
                keep = waits[-max_waits:]
                spill = waits[:-max_waits]
                for i in range(0, len(spill), max_waits):
                    nop = mybir.InstNoOp(name=f"WSPILL-{n}", ins=[], outs=[])
                    n += 1
                    nop.engine = inst.engine
                    nop.sync_info = mybir.SyncInfo(
                        on_wait=spill[i : i + max_waits], on_update=[]
                    )
                    nc.register_instruction(nop, overwrite=True)
                    out.append(nop)
                inst.sync_info = mybir.SyncInfo(
                    on_wait=keep, on_update=list(si.on_update)
                )
                changed = True
            out.append(inst)
        if changed:
            bb.instructions = out
    return n


def build():
    nc = bass.Bass()
    # x/w arrive pre-tiled from the host ([partition][chunk][k-sub][cols])
    # so every DMA descriptor is 4-8KB contiguous per partition instead of
    # the 1KB slivers a strided rearrange would produce (~3x DMA speedup
    # on the 16MB input stream; the startup matmul gate is DMA-bound).
    xq = nc.dram_tensor("xq", [P, NCH, ND, SC], BF16, kind="ExternalInput")
    xk = nc.dram_tensor("xk", [P, NCH, ND, SC], BF16, kind="ExternalInput")
    xv = nc.dram_tensor("xv", [P, NCH, ND, SC], BF16, kind="ExternalInput")
    wq = nc.dram_tensor("wq", [P, ND, DL], BF16, kind="ExternalInput")
    wk = nc.dram_tensor("wk", [P, ND, DL], BF16, kind="ExternalInput")
    wv = nc.dram_tensor("wv", [P, ND, DL], BF16, kind="ExternalInput")
    wo = nc.dram_tensor("wo", [P, NM, D], BF16, kind="ExternalInput")
    bq = nc.dram_tensor("bq", [DL], F32, kind="ExternalInput")
    bk = nc.dram_tensor("bk", [DL], F32, kind="ExternalInput")
    bv = nc.dram_tensor("bv", [DL], F32R, kind="ExternalInput")
    out = nc.dram_tensor("out", [S, D], BF16, kind="ExternalOutput")

    xq_r, xk_r, xv_r = xq, xk, xv
    wq_r, wk_r, wv_r, wo_r = wq, wk, wv, wo
    out_r = out.rearrange("(so p) n -> p so n", p=P)

    with TileContext(nc) as tc:
        with (
            tc.tile_pool(name="persist", bufs=1) as persist,
            tc.tile_pool(name="wpool", bufs=3) as wpool,
            tc.tile_pool(name="xpool", bufs=10) as xpool,
            tc.tile_pool(name="expp", bufs=4) as expp,
            tc.tile_pool(name="attnp", bufs=2) as attnp,
            tc.tile_pool(name="osb", bufs=2) as osbp,
            tc.tile_pool(name="nrm", bufs=2) as nrm,
            tc.tile_pool(name="ps_big", bufs=1, space="PSUM") as ps_big,
            tc.tile_pool(name="ps_pv", bufs=1, space="PSUM") as ps_pv,
            tc.tile_pool(name="ps_sm", bufs=2, space="PSUM") as ps_sm,
        ):
            qt = persist.tile([P, NM, S], BF16, tag="qt")
            kt = persist.tile([P, NM, S], BF16, tag="kt")
            vaug = persist.tile([P, NT, HL, 66], BF16, tag="vaug")
            wq_t = persist.tile([P, ND, DL], BF16, tag="wq")
            ones_f = persist.tile([P, P], F32, tag="ones_f")
            ones_r = persist.tile([P, P], F32R, tag="ones_r")
            bq_sb = persist.tile([P, NM], F32, tag="bq")
            bk_sb = persist.tile([P, NM], F32, tag="bk")
            bv_t = persist.tile([P, DL], F32R, tag="bv_t")
            bv_bc = persist.tile([P, DL], F32, tag="bv_bc")

            # ---- constants / biases ----
            nc.vector.memset(ones_f[:], 1.0)
            nc.vector.tensor_copy(ones_r[:], ones_f[:])
            nc.sync.dma_start(bq_sb[:], bq.rearrange("(o p) -> p o", p=P))
            nc.sync.dma_start(bk_sb[:], bk.rearrange("(o p) -> p o", p=P))
            nc.sync.dma_start(bv_t[0:1, :], bv[None, :])
            ps = ps_sm.tile([P, SC], F32, tag="sm")
            nc.tensor.matmul(ps[:], ones_r[0:1, 0:P], bv_t[0:1, :], start=True,
                             stop=True)
            nc.vector.tensor_copy(bv_bc[:], ps[:])
            # ones columns of V_aug
            of = ones_f[:, 0:NT * HL].rearrange("p (a b) -> p a b", a=NT)
            nc.vector.tensor_copy(vaug[:, :, :, 0:1], of[:, :, :, None])
            nc.vector.tensor_copy(vaug[:, :, :, 65:66], of[:, :, :, None])

            def load_x_chunk(x_r, c):
                xa = xpool.tile([P, ND // 2, SC], BF16, tag="x")
                xb = xpool.tile([P, ND // 2, SC], BF16, tag="x")
                nc.sync.dma_start(xa[:], x_r[:, c, 0 : ND // 2, :])
                nc.sync.dma_start(xb[:], x_r[:, c, ND // 2 : ND, :])
                return xa, xb

            def proj_chunk_steps(dst, w_tile, x_r, c, bias_sb, xs=None):
                """Generator: each step emits ~2 matmuls (or 2 bias adds) so
                the attention loop can trickle projection work into its
                per-group PE slack instead of a blocking burst."""
                halves = xs if xs is not None else load_x_chunk(x_r, c)
                # pairs of m-tiles with interleaved k-chains: consecutive
                # matmuls accumulate into different psum banks (no RMW stall)
                for i in range(NM // 2):
                    psms = (ps_sm.tile([P, SC], F32, tag="sm", name="psm0"),
                            ps_sm.tile([P, SC], F32, tag="sm", name="psm1"))
                    for k in range(ND):
                        for h, psm in enumerate(psms):
                            m = 2 * i + h
                            nc.tensor.matmul(
                                psm[:],
                                w_tile[:, k, m * P : (m + 1) * P],
                                halves[k // 4][:, k % 4, :],
                                start=(k == 0),
                                stop=(k == ND - 1),
                            )
                        yield
                    for h, psm in enumerate(psms):
                        m = 2 * i + h
                        nc.vector.tensor_add(
                            dst[:, m, c * SC : (c + 1) * SC],
                            psm[:],
                            bias_sb[:, m : m + 1].to_broadcast((P, SC)),
                        )
                    yield

            def proj_chunk(dst, w_tile, x_r, c, bias_sb, xs=None):
                for _ in proj_chunk_steps(dst, w_tile, x_r, c, bias_sb, xs):
                    pass

            def vproj_chunk_steps(wv_t, c, xa, xb):
                halves = (xa, xb)
                for i2 in range(2):
                    psms = (ps_sm.tile([P, SC], F32, tag="sm", name="psm0"),
                            ps_sm.tile([P, SC], F32, tag="sm", name="psm1"))
                    for k in range(ND):
                        for h2, psm in enumerate(psms):
                            i = 2 * i2 + h2
                            nc.tensor.matmul(
                                psm[:],
                                halves[k // 4][:, k % 4, i * P : (i + 1) * P],
                                wv_t[:, k, :],
                                start=(k == 0),
                                stop=(k == ND - 1),
                            )
                        yield
                    for h2, psm in enumerate(psms):
                        t_o = c * 4 + 2 * i2 + h2
                        for h in range(HL):
                            nc.vector.tensor_add(
                                vaug[:, t_o, h, 1:65],
                                psm[:, h * DH : (h + 1) * DH],
                                bv_bc[:, h * DH : (h + 1) * DH],
                            )
                    yield

            def normalize_pre(pv):
                """Consume the PV psum right away (frees the psum slot): copy
                unnormalized rows on DVE; 1/denom = exp(-ln(denom)) on ACT
                (DVE reciprocal is an 8-pass iterative divide; ln and exp
                share one act table set so these cost ~1.1us each). pv is
                [P, 2, SC] (banks e/o); both denoms sit on partition 64 so
                the copy/ln/exp each run as ONE op over both banks."""
                raw = nrm.tile([P, 2, SC], F32, tag="raw")
                rec = nrm.tile([P, 2, SC], F32R, tag="rec")
                nc.vector.tensor_copy(raw[0:64, :, :], pv[0:64, :, :])
                nc.scalar.activation(raw[64:65, :, :], pv[64:65, :, :], AF.Ln)
                nc.scalar.activation(rec[64:65, :, :], raw[64:65, :, :],
                                     AF.Exp, scale=-1.0)
                return raw, rec

            def normalize_post(raw, rec, p, att):
                """Broadcast 1/denom across partitions (K=1 matmul) and apply."""
                bc_e = ps_sm.tile([P, SC], F32, tag="sm")
                nc.tensor.matmul(bc_e[0:64, :], ones_r[64:65, 0:64],
                                 rec[64:65, 0, :], start=True, stop=True)
                nc.vector.tensor_mul(att[0:64, p, :], bc_e[0:64, :],
                                     raw[0:64, 0, :])
                bc_o = ps_sm.tile([P, SC], F32, tag="sm")
                nc.tensor.matmul(bc_o[0:64, :], ones_r[64:65, 0:64],
                                 rec[64:65, 1, :], start=True, stop=True)
                tmp = nrm.tile([P, SC], BF16, tag="tmp")
                nc.vector.tensor_mul(tmp[0:64, :], bc_o[0:64, :],
                                     raw[0:64, 1, :])
                # scalar-DGE queue: keeps this small partition-shift transfer
                # from queueing behind megabyte output DMAs on the SP queue
                # (it gates the next chunk's o-proj start)
                nc.scalar.dma_start(att[64:128, p, :], tmp[0:64, :])

            # ---- projections (prefix) ----
            # q-chunk-0 x DMAs lead the queue (they gate the very first
            # matmul); weight DMAs follow (wpool bufs=3 keeps k/v/o
            # resident together; bf16 makes that cheap).
            xq0a = xpool.tile([P, ND // 2, SC], BF16, tag="x", name="xq0a")
            xq0b = xpool.tile([P, ND // 2, SC], BF16, tag="x", name="xq0b")
            nc.sync.dma_start(xq0a[:], xq_r[:, 0, 0 : ND // 2, :])
            nc.sync.dma_start(wq_t[:, 0 : ND // 2], wq_r[:, 0 : ND // 2])
            nc.sync.dma_start(xq0b[:], xq_r[:, 0, ND // 2 : ND, :])
            nc.sync.dma_start(wq_t[:, ND // 2 :], wq_r[:, ND // 2 :])
            xq0 = (xq0a, xq0b)
            wk_t = wpool.tile([P, ND, DL], BF16, tag="w")
            nc.sync.dma_start(wk_t[:, 0 : ND // 2], wk_r[:, 0 : ND // 2])
            nc.sync.dma_start(wk_t[:, ND // 2 :], wk_r[:, ND // 2 :])
            # all xk chunks queue before wv/wo (k-proj consumes them long
            # before V/O weights are touched)
            kx = [load_x_chunk(xk_r, c) for c in range(NCH)]
            proj_chunk(qt, wq_t, xq_r, 0, bq_sb, xs=xq0)
            wv_t = wpool.tile([P, ND, DL], BF16, tag="w")
            nc.sync.dma_start(wv_t[:], wv_r[:])
            wo_t = wpool.tile([P, NM, D], BF16, tag="w")
            nc.sync.dma_start(wo_t[:], wo_r[:])
            for c in range(NCH):
                proj_chunk(kt, wk_t, xk_r, c, bk_sb, xs=kx[c])
            # issue all xv DMAs now (xpool bufs=10 holds them) so no vproj
            # step ever head-of-line blocks the PE on a transfer
            vx = [load_x_chunk(xv_r, c) for c in range(NCH)]
            vproj_gens = [
                vproj_chunk_steps(wv_t, c, *vx[c]) for c in range(NCH)
            ]

            # ---- attention + o-proj, per s-chunk ----
            def oproj_chunk_steps(c, attn_t):
                for st in range(4):
                    psms = (ps_sm.tile([P, SC], F32, tag="sm", name="psm0"),
                            ps_sm.tile([P, SC], F32, tag="sm", name="psm1"))
                    for do in range(NM):
                        for n, psm in enumerate(psms):
                            nc.tensor.matmul(
                                psm[:],
                                attn_t[:, do, st * P : (st + 1) * P],
                                wo_t[:, do, n * SC : (n + 1) * SC],
                                start=(do == 0),
                                stop=(do == NM - 1),
                            )
                        yield
                    for n, psm in enumerate(psms):
                        ob = osbp.tile([P, SC], BF16, tag="ob")
                        nc.vector.tensor_copy(ob[:], psm[:])
                        nc.sync.dma_start(
                            out_r[:, c * 4 + st, n * SC : (n + 1) * SC], ob[:]
                        )
                    yield

            def oproj_chunk(c, attn_t):
                for _ in oproj_chunk_steps(c, attn_t):
                    pass

            def emit_pv_half(pend, half):
                # consecutive PVs alternate psum banks (e,o,e,o) so the
                # accumulate RMW turnaround never hits a back-to-back bank
                ex_a, ex_b, ta, tb, p, pv, att = pend
                ex, t = (ex_a, ta) if half == 0 else (ex_b, tb)
                for hh, bank in ((2 * p, 0), (2 * p + 1, 1)):
                    nc.tensor.matmul(
                        pv[0:65, bank, :],
                        vaug[:, t, hh, 1:66],
                        ex[:, hh % 2, :],
                        start=(t == 0),
                        stop=(t == NT - 1),
                    )

            def finish_pend(pend):
                """After both PV halves are out: when that group was its p's
                last, normalize and return the deferred normalize_post
                closure for that p."""
                ex_a, ex_b, ta, tb, p, pv, att = pend
                if tb != NT - 1:
                    return None
                raw, rec = normalize_pre(pv)

                def run(raw=raw, rec=rec, p=p, att=att):
                    normalize_post(raw, rec, p, att)

                return run

            def flush_pend(pend):
                emit_pv_half(pend, 0)
                emit_pv_half(pend, 1)
                return finish_pend(pend)

            # flat software pipeline across (c, p, g): `pend` (the PV work
            # for the previous score group) carries across p- and c-
            # boundaries, so the PE never drains while a p finishes up
            # (PV flush + normalize happen AFTER the next p's first scores).
            # Projection work (next-chunk qproj, prev-chunk oproj) trickles
            # in as ~2-matmul background steps per group slot instead of
            # multi-us bursts that would stall the scores->exp chain.
            bg = collections.deque()

            def bg_pull(steps=None):
                if steps is None:
                    steps = 2 if len(bg) > 1 else 1
                while bg and steps > 0:
                    try:
                        next(bg[0])
                        steps -= 1
                    except StopIteration:
                        bg.popleft()

            for gen in vproj_gens:
                bg.append(gen)
            pends = collections.deque()
            deferred = None
            prev_attn = None
            for c in range(NCH):
                attn_t = attnp.tile([P, NM, SC], BF16, tag="attn")
                cs = slice(c * SC, (c + 1) * SC)
                for p in range(NM):
                    pv = ps_pv.tile([P, 2, SC], F32, tag="pv")
                    # during (c0, p0) V-proj streams in via bg; delay PV
                    # flushing by one extra group so each PV's vaug tiles
                    # are already emitted ahead of it in PE program order.
                    warmup = c == 0 and p == 0
                    depth = 2 if warmup else 1
                    for g in range(NT // 2):
                        ta, tb = 2 * g, 2 * g + 1
                        # separate a/b tiles so every scores->exp->PV dep is
                        # tile-atomic: shared-tile subtile deps coalesced into
                        # cross-group semaphore waits (PV(g) stalling on
                        # exp#1(g+1)) that serialized the whole slot.
                        big_a = ps_big.tile([P, 2, SC], F32, tag="biga")
                        big_b = ps_big.tile([P, 2, SC], F32, tag="bigb")
                        ex_a = expp.tile([P, 2, SC], BF16, tag="exa")
                        ex_b = expp.tile([P, 2, SC], BF16, tag="exb")
                        flush = None
                        if INTERLEAVE:
                            # slot order [s0 s1][PVa][s2 s3][PVb]: the prior
                            # group's PV halves sit between this group's
                            # score pairs so same-bank PV accumulates land
                            # 3+ instructions apart (RMW turnaround hides)
                            while len(pends) > depth:
                                prev = pends.popleft()
                                if flush is not None:
                                    d = flush_pend(flush)
                                    if d is not None:
                                        deferred = d
                                flush = prev
                        for h, lo in enumerate((0, 64)):
                            nc.tensor.matmul(
                                big_a[:, h, :],
                                kt[lo : lo + 64, p, ta * P : (ta + 1) * P],
                                qt[lo : lo + 64, p, cs],
                                start=True, stop=True,
                            )
                        nc.scalar.activation(ex_a[:], big_a[:], AF.Exp,
                                             scale=0.125)
                        if flush is not None:
                            emit_pv_half(flush, 0)
                        for h, lo in enumerate((0, 64)):
                            nc.tensor.matmul(
                                big_b[:, h, :],
                                kt[lo : lo + 64, p, tb * P : (tb + 1) * P],
                                qt[lo : lo + 64, p, cs],
                                start=True, stop=True,
                            )
                        nc.scalar.activation(ex_b[:], big_b[:], AF.Exp,
                                             scale=0.125)
                        if flush is not None:
                            emit_pv_half(flush, 1)
                            d = finish_pend(flush)
                            if d is not None:
                                deferred = d
                        if not INTERLEAVE and len(pends) >= depth + 1:
                            # flush two groups' PVs together on even slots,
                            # bg-proj bursts on odd slots: every matmul-type
                            # switch (scores<->PV<->proj weight geometry)
                            # costs the PE ~120ns of pipeline turnaround, so
                            # batch same-type runs
                            while pends:
                                d = flush_pend(pends.popleft())
                                if d is not None:
                                    deferred = d
                        if warmup:
                            # one V-proj pair (2 t-tiles) per slot = 9 steps,
                            # emitted before the PV that will consume it
                            for _ in range(9):
                                bg_pull()
                        if g == 3 and deferred is not None:
                            deferred()
                            deferred = None
                        if g == 5 and p == 0 and prev_attn is not None:
                            bg.append(oproj_chunk_steps(c - 1, prev_attn))
                            prev_attn = None
                        if g == 0 and p == 1 and c < NCH - 1:
                            bg.append(proj_chunk_steps(qt, wq_t, xq_r, c + 1,
                                                       bq_sb))
                        pends.append((ex_a, ex_b, ta, tb, p, pv, attn_t))
                        if not warmup:
                            if INTERLEAVE:
                                bg_pull(2 if len(bg) > 1 else 1)
                            elif g % 2 == 1:
                                # deeper bursts while the queue is long:
                                # ACT-bound slots have PE slack to absorb
                                # extra proj work
                                bg_pull(3 if len(bg) > 1 else 2)
                prev_attn = attn_t
            while bg:
                bg_pull()
            while pends:
                d = flush_pend(pends.popleft())
                if d is not None:
                    if deferred is not None:
                        deferred()
                    deferred = d
            deferred()
            oproj_chunk(NCH - 1, prev_attn)

    _split_excess_waits(nc)
    return nc


_CACHE = {}


def _get_nc():
    if "nc" not in _CACHE:
        _CACHE["nc"] = build()
    return _CACHE["nc"]


def _f32(x):
    return np.asarray(x).astype(np.float32, copy=False)


def _tile_x(xt):
    # [D, S] -> [P, NCH, ND, SC]: per-(partition, chunk) contiguous 4KB runs
    return np.ascontiguousarray(
        xt.reshape(ND, P, NCH, SC).transpose(1, 2, 0, 3)
    )


def _tile_w(wt):
    # [D, M] -> [P, ND, M]
    return np.ascontiguousarray(
        wt.reshape(wt.shape[0] // P, P, wt.shape[1]).transpose(1, 0, 2)
    )


def _prep_core_inputs(c, q, k, v, w_q, b_q, w_k, b_k, w_v, b_v, w_o, b_o):
    b, hg = c // 2, c % 2
    hs = slice(hg * DL, hg * DL + DL)
    bf = ml_dtypes.bfloat16
    return {
        "xq": _tile_x(q[b].T.astype(bf)),
        "xk": _tile_x(k[b].T.astype(bf)),
        "xv": _tile_x(v[b].T.astype(bf)),
        "wq": _tile_w(w_q[hs, :].T.astype(bf)),
        "wk": _tile_w(w_k[hs, :].T.astype(bf)),
        "wv": _tile_w(w_v[hs, :].T.astype(bf)),
        "wo": _tile_w(w_o[:, hs].T.astype(bf)),
        "bq": np.ascontiguousarray(b_q[hs]),
        "bk": np.ascontiguousarray(b_k[hs]),
        "bv": np.ascontiguousarray(b_v[hs]),
    }


def kernel(q, k, v, w_q, b_q, w_k, b_k, w_v, b_v, w_o, b_o):
    q, k, v = _f32(q), _f32(k), _f32(v)
    w_q, b_q = _f32(w_q), _f32(b_q)
    w_k, b_k = _f32(w_k), _f32(b_k)
    w_v, b_v = _f32(w_v), _f32(b_v)
    w_o, b_o = _f32(w_o), _f32(b_o)

    nc = _get_nc()
    in_maps = [
        _prep_core_inputs(c, q, k, v, w_q, b_q, w_k, b_k, w_v, b_v, w_o, b_o)
        for c in range(8)
    ]
    res = run_bass_kernel_spmd(nc, in_maps, core_ids=list(range(8)))
    out = np.empty((B, S, D), np.float32)
    for b in range(B):
        out[b] = (
            res.results[2 * b]["out"].astype(np.float32)
            + res.results[2 * b + 1]["out"].astype(np.float32)
            + b_o
        )
    return out

